# revision 92
# baseline (speedup 1.0000x reference)
"""Trainium2 Bass kernel for nn_Attention_91293824843977.

Self-contained: hardcodes shapes B=8, T=64, N=196, C=384, H=6 and shards
batch-parallel across 8 NeuronCores (one batch element per core).

Design (in rough order of impact):
 - qkv projections run in fp8 DoubleRow (x*4 and W*64 quantized to
   fp8-e4m3, shipped as uint8 and bitcast; drains rescale by 2^-8); the
   odd third c-chunk is DoubleRow-paired with a zero/ones pad chunk.
   Scores/AV/out-proj stay bf16 to hold rel err under the 2e-2 gate.
 - qk is computed one block-pair ahead, its head-pair groups interleaved
   with the current pair's stages; elementwise drains are emitted one
   stage after their matmuls.
 - the output projection is globalized over exact 128-token chunks that
   span block boundaries via a persistent SBUF ring (6272 tokens =
   lcm(196,128)), eliminating partial-chunk waste; chunk matmuls fire
   one block late and results store as a flat bf16 token stream.
 - PSUM: 3x 2-bank slots (scores per head-pair, v, av, yT transposes)
   + 2x 1-bank (qk projection, out chunks); v bias is folded into the
   projection bias host-side (bp' = bv@WpT_perm + bp).
 - all DMA on the SP ring (keeps Act's sequencer free for exp/drains).
"""

import sys

for p in ("/opt/trn_rl_repo", "/root/.axon_site/_ro/trn_rl_repo"):
    if p not in sys.path:
        sys.path.append(p)

import numpy as np
import ml_dtypes

import concourse.bass as bass
import concourse.mybir as mybir
import concourse.tile as tile

BF16 = mybir.dt.bfloat16
F32 = mybir.dt.float32
FP8 = mybir.dt.float8e4
U8 = mybir.dt.uint8
QK_DESCALE = 2.0 ** -8
AF = mybir.ActivationFunctionType
ALU = mybir.AluOpType

EMBED = 384
NH = 6
HD = 64
NP = 3  # head pairs
N = 196  # tokens per block
SCALE = HD ** -0.5

# m (key token) chunking: 196 = 128 + 68
MC = [(0, 128), (128, 68)]

# engine assignment for the qk psum drains, per head-pair (tunable)
QK_DRAIN_ENGINE = ["act", "mix", "act"]


def prep_inputs(x, W_qkv, b_qkv, W_proj, b_proj):
    """Host-side prep: cast/transpose/permute weights, shard x by batch.

    Returns (common, xs) where xs[b] is the per-core x input.
    """
    B = x.shape[0]
    T = x.shape[1]
    bf16 = ml_dtypes.bfloat16

    # x pre-transposed per block-pair: [T/2, 3(c-chunk), 128(c), 392(tok)]
    xT_h = x.reshape(B, T // 2, 2 * N, EMBED).transpose(0, 1, 3, 2)  # [B,TP,C,2N]

    fp8 = ml_dtypes.float8_e4m3

    Wq = W_qkv[0:384] * SCALE  # [384(o=h*64+d), 384(c)]
    Wk = W_qkv[384:768]
    Wv = W_qkv[768:1152]
    WqkT = np.concatenate([Wq, Wk], axis=0).T  # [384 c, 768 o]

    # fp8 v weights scaled by 64, DoubleRow-paired c-chunks:
    # [2(grp: c01, c2+zero), 128(k), 2(i), 384(o)]
    WvT = Wv.T * 64.0  # [384 c, 384 o]
    wv8 = np.zeros((2, 128, 2, 384), dtype=np.float32)
    wv8[0, :, 0, :] = WvT[0:128]
    wv8[0, :, 1, :] = WvT[128:256]
    wv8[1, :, 0, :] = WvT[256:384]
    wv8 = np.ascontiguousarray(wv8.astype(fp8)).view(np.uint8)

    # fp8 x (scaled by 4) in the same transposed layout as xT
    x8_h = np.ascontiguousarray((xT_h * 4.0).reshape(
        B, T // 2, 3, 128, 2 * N).astype(fp8)).view(np.uint8)

    # fp8 qk weights scaled by 64 (drains rescale by 2^-8):
    # DoubleRow pairs c-chunks 0/1: [128(k), 6(j), 2(i), 128(m)]
    Ws = WqkT * 64.0  # [384 c, 768 o]
    wqk8_dr = np.empty((128, 6, 2, 128), dtype=np.float32)
    for j in range(6):
        for i in range(2):
            wqk8_dr[:, j, i, :] = Ws[
                i * 128: i * 128 + 128, j * 128: j * 128 + 128
            ]
    wqk8_dr = np.ascontiguousarray(wqk8_dr.astype(fp8)).view(np.uint8)
    # c-chunk 2 also runs DoubleRow, paired with a zero chunk:
    # [128(k), 6(j), 2(i), 128(m)], i=1 half zero
    wqk8_c2 = np.zeros((128, 6, 2, 128), dtype=np.float32)
    wqk8_c2[:, :, 0, :] = Ws[256:384].reshape(128, 6, 128)
    wqk8_c2 = np.ascontiguousarray(wqk8_c2.astype(fp8)).view(np.uint8)

    # proj: y channel c' = h*64+d maps to original channel d*6+h
    perm = np.array([d * NH + h for h in range(NH) for d in range(HD)])
    WpT_perm = W_proj.T[perm, :]  # [384 c', 384 o]
    wp = np.ascontiguousarray(WpT_perm.reshape(3, 128, 384).astype(bf16))

    bqk = np.concatenate([b_qkv[0:384] * SCALE, b_qkv[384:768]])
    bqk = np.ascontiguousarray(bqk.reshape(6, 128).T.astype(np.float32))  # [128, 6]
    # v bias folded through the projection: out += bv @ WpT_perm.
    # bv is indexed by o = h*64+d which matches WpT_perm's (permuted) rows.
    bv = b_qkv[768:1152]
    bp_eff = bv @ WpT_perm + b_proj  # [384]
    bp = np.ascontiguousarray(bp_eff.reshape(1, 384).astype(np.float32))

    ident = np.eye(128, dtype=bf16)
    common = {
        "wqk8_dr": wqk8_dr, "wqk8_c2": wqk8_c2, "wv8": wv8, "wp": wp,
        "bqk": bqk, "bp": bp, "ident": ident,
    }
    xs = [{"x8": x8_h[b]} for b in range(B)]
    return common, xs


def declare_tensors(nc, T):
    t = {}
    t["x8"] = nc.dram_tensor(
        "x8", [T // 2, 3, 128, 2 * N], U8, kind="ExternalInput"
    ).ap()
    t["wqk8_dr"] = nc.dram_tensor(
        "wqk8_dr", [128, 6, 2, 128], U8, kind="ExternalInput"
    ).ap()
    t["wqk8_c2"] = nc.dram_tensor(
        "wqk8_c2", [128, 6, 2, 128], U8, kind="ExternalInput"
    ).ap()
    t["wv8"] = nc.dram_tensor(
        "wv8", [2, 128, 2, 384], U8, kind="ExternalInput"
    ).ap()
    t["wp"] = nc.dram_tensor("wp", [3, 128, 384], BF16, kind="ExternalInput").ap()
    t["bqk"] = nc.dram_tensor("bqk", [128, 6], F32, kind="ExternalInput").ap()
    t["bp"] = nc.dram_tensor("bp", [1, 384], F32, kind="ExternalInput").ap()
    t["ident"] = nc.dram_tensor("ident", [128, 128], BF16, kind="ExternalInput").ap()
    # bf16, flat token stream [T*N, 384], stored in 896-token groups
    t["out"] = nc.dram_tensor(
        "out", [T * N, EMBED], BF16, kind="ExternalOutput"
    ).ap()
    return t


def build(tc, t, T):
    """Emit the Tile program. t: dict of DRAM APs, T: number of blocks."""
    nc = tc.nc

    import contextlib

    ctx = contextlib.ExitStack()
    with ctx:
        singles = ctx.enter_context(tc.tile_pool(name="singles", bufs=1))
        sbx = ctx.enter_context(tc.tile_pool(name="sbx", bufs=8))
        sbqk = ctx.enter_context(tc.tile_pool(name="sbqk", bufs=3))
        sbattn = ctx.enter_context(tc.tile_pool(name="sbattn", bufs=12))
        sbv = ctx.enter_context(tc.tile_pool(name="sbv", bufs=5))
        sby = ctx.enter_context(tc.tile_pool(name="sby", bufs=3))
        sbout = ctx.enter_context(tc.tile_pool(name="sbout", bufs=3))
        sbr = ctx.enter_context(tc.tile_pool(name="sbr", bufs=4))
        # PSUM: 3 x 2-bank slots (sc/v/av/o) + 2 x 1-bank (qk, transposes)
        ps2 = ctx.enter_context(tc.tile_pool(name="ps2", bufs=3, space="PSUM"))
        ps1 = ctx.enter_context(tc.tile_pool(name="ps1", bufs=2, space="PSUM"))

        # warm the Act exp-table at t=0 (off the critical path); the 1-elem
        # result lands in a scratch tile and is never read
        warm = singles.tile([1, 2], F32)
        nc.scalar.activation(warm[0:1, 1:2], warm[0:1, 0:1], AF.Exp)

        # --- constants / weights resident in SBUF ---
        # wqk8_c2 (used by the very first matmul) loads first; x8(0) queues
        # right behind it via qk_start(0), then the remaining weights
        wqk8_c2_sb = singles.tile([128, 6, 2, 128], FP8)
        nc.sync.dma_start(wqk8_c2_sb, t["wqk8_c2"].bitcast(FP8))
        wqk8_dr_sb = singles.tile([128, 6, 2, 128], FP8)
        bqk_sb = singles.tile([128, 6], F32)

        def load_early_consts():
            nc.sync.dma_start(wqk8_dr_sb, t["wqk8_dr"].bitcast(FP8))
            nc.sync.dma_start(bqk_sb, t["bqk"])

        out2 = t["out"]

        # rotating slot state for zero-padding qk_sb pad columns once
        padded_qk_slots = [False] * 3
        qk_slot_idx = [0]
        zeroed_x8_slots = [False] * 8
        x8_slot_idx = [0]
        late_consts = {}

        def load_late_consts():
            wv8_sb = singles.tile([128, 2, 2, 384], FP8)
            nc.sync.dma_start(wv8_sb, t["wv8"].rearrange("g p i o -> p g i o").bitcast(FP8))
            wp_sb = singles.tile([128, 3, 384], BF16)
            nc.sync.dma_start(wp_sb, t["wp"].rearrange("k p o -> p k o"))
            ident = singles.tile([128, 128], BF16)
            nc.sync.dma_start(ident, t["ident"])
            # per-partition broadcast of the out bias (stride-0 DRAM re-read)
            bp_bc = singles.tile([128, 384], F32)
            nc.sync.dma_start(
                bp_bc,
                bass.AP(tensor=t["bp"].tensor, offset=0, ap=[[0, 128], [1, 384]]),
            )
            # prime PE's view of the weight DMAs
            nc.tensor.ldweights(weights=wqk8_c2_sb[:, 0, 0, :])

            nc.tensor.ldweights(weights=wp_sb[:, 0, 0:128])
            late_consts.update(wv8_sb=wv8_sb, wp_sb=wp_sb, ident=ident, bp_bc=bp_bc)

        def qk_start(tp):
            """Allocate x8/qk_sb for pair tp, issue the load."""
            # free-dim stride 400 (mult of 16) for the DoubleRow pair dim;
            # 4th chunk stays zero so c-chunk 2 can DoubleRow-pair with it
            x8 = sbx.tile([128, 4, 400], FP8, name="x8", tag="x8")
            sl8 = x8_slot_idx[0] % 8
            x8_slot_idx[0] += 1
            if not zeroed_x8_slots[sl8]:
                zeroed_x8_slots[sl8] = True
                nc.gpsimd.memset(x8[:, 3, :], 0.0)
            nc.sync.dma_start(
                x8[:, 0:3, 0:392],
                t["x8"][tp].rearrange("k p n -> p k n").bitcast(FP8),
            )

            qk_sb = sbqk.tile([128, 6, 2, 256], BF16, name="qk_sb")
            sl = qk_slot_idx[0] % 3
            qk_slot_idx[0] += 1
            if not padded_qk_slots[sl]:
                padded_qk_slots[sl] = True
                # zero the 196:256 pad once per slot (gpsimd, off critical path)
                nc.gpsimd.memset(qk_sb[:, :, :, 196:256], 0.0)
            return x8, qk_sb

        def qk_mms(x8, p):
            """qk matmuls for head-pair p -> two 1-bank PSUM tiles."""
            pss = []
            for j2, j in enumerate((p, 3 + p)):
                qk_ps = ps1.tile([128, 512], F32, tag="ps1", name="qk_ps")
                nc.tensor.matmul(
                    qk_ps[:, 0:392],
                    wqk8_c2_sb[:, j, :, :],
                    x8[:, 2:4, 0:392],
                    start=True,
                    stop=False,
                    perf_mode=mybir.MatmulPerfMode.DoubleRow,
                    skip_group_check=True,
                )
                nc.tensor.matmul(
                    qk_ps[:, 0:392],
                    wqk8_dr_sb[:, j, :, :],
                    x8[:, 0:2, 0:392],
                    start=False,
                    stop=True,
                    perf_mode=mybir.MatmulPerfMode.DoubleRow,
                    skip_group_check=True,
                )
                pss.append(qk_ps)
            return pss

        def qk_drain(qk_sb, p, pss):
            """descale+bias drains for head-pair p (emitted one stage after
            the matmuls so Act/DVE never head-of-line block on them)."""
            for j2, j in enumerate((p, 3 + p)):
                qk_ps = pss[j2]
                eng = QK_DRAIN_ENGINE[p]
                if eng == "mix":
                    eng = "act" if j2 == 0 else "dve"
                if eng == "act":
                    nc.scalar.activation(
                        qk_sb[:, j, :, 0:196],
                        qk_ps[:, 0:392].rearrange("p (b n) -> p b n", n=196),
                        AF.Identity,
                        bias=bqk_sb[:, j: j + 1],
                        scale=QK_DESCALE,
                    )
                else:
                    nc.vector.tensor_scalar(
                        qk_sb[:, j, :, 0:196],
                        qk_ps[:, 0:392].rearrange("p (b n) -> p b n", n=196),
                        QK_DESCALE,
                        bqk_sb[:, j: j + 1],
                        ALU.mult,
                        ALU.add,
                    )

        def v_proj(tb, x8, bi):
            """v projection for block tb -> v_sb [128(tok), 2(mc), 6, 65].

            fp8 DoubleRow: x8-pair is stationary (out tokens start at
            partition 0 for both m-chunks, as DoubleRow requires); drain
            applies the 2^-8 descale."""
            wv8_sb = late_consts["wv8_sb"]
            v_sb = sbv.tile([128, 2, 6, 65], BF16, name="v_sb")
            nc.gpsimd.memset(v_sb[:, :, :, 64:65], 1.0)
            v_ps = ps2.tile([128, 2, 512], F32, tag="ps2", name="v_ps")
            for mc, (m0, ml) in enumerate(MC):
                t0 = bi * 196 + m0
                for g in range(2):
                    nc.tensor.matmul(
                        v_ps[0:ml, mc, 0:384],
                        x8[:, 2 * g: 2 * g + 2, t0: t0 + ml],
                        wv8_sb[:, g, :, :],
                        start=(g == 0),
                        stop=(g == 1),
                        perf_mode=mybir.MatmulPerfMode.DoubleRow,
                        skip_group_check=True,
                    )
            return v_sb, v_ps

        def v_drain(v_sb, v_ps):
            # single fused descale drain (rows 68:128 of mc1 junk, unread)
            nc.vector.tensor_scalar_mul(
                v_sb[:, :, :, 0:64],
                v_ps[:, :, 0:384].rearrange("p m (h d) -> p m h d", d=64),
                QK_DESCALE,
            )

        def scores(tb, qk_sb, bi):
            """scores+exp for block tb -> list of attn tiles per head-pair.

            attn_hp [128(m), 2(h2), 392]: cols 0:196 = m-chunk0 (m=0:128),
            cols 196:392 = m-chunk1 (m=0:68; rows 68: junk exp(0)).
            """
            attns = []
            for p in range(NP):
                sc = ps2.tile([128, 2, 512], F32, tag="ps2", name="sc")
                for h2 in range(2):
                    r = slice(h2 * 64, h2 * 64 + 64)
                    kT = qk_sb[:, 3 + p, bi, :]
                    qT = qk_sb[:, p, bi, 0:196]
                    nc.tensor.matmul(
                        sc[:, h2, 0:196],
                        kT[r, 0:128],
                        qT[r, :],
                        start=True,
                        stop=True,
                        tile_position=(h2 * 64, 0),
                    )
                    nc.tensor.matmul(
                        sc[:, h2, 196:392],
                        kT[r, 128:256],
                        qT[r, :],
                        start=True,
                        stop=True,
                        tile_position=(h2 * 64, 0),
                    )
                attn = sbattn.tile([128, 2, 392], BF16, name="attn")
                nc.scalar.activation(
                    attn,
                    sc[:, :, 0:392],
                    AF.Exp,
                )
                attns.append(attn)
            return attns

        def av_stage(tb, attns, v_sb):
            """AV with denominator; normalize -> y_sb [128, 2(nc), 6, 64]."""
            y_sb = sbr.tile([128, 2, 6, 64], BF16, name="y_sb")
            av_ps = ps2.tile([128, 2, 512], F32, tag="ps2", name="av_ps")
            for nc_i, (n0, nl) in enumerate(MC):
                for p in range(NP):
                    for h2 in range(2):
                        h = 2 * p + h2
                        for mc, (m0, ml) in enumerate(MC):
                            nc.tensor.matmul(
                                av_ps[0:nl, nc_i, h * 65: h * 65 + 65],
                                attns[p][0:ml, h2, mc * 196 + n0: mc * 196 + n0 + nl],
                                v_sb[0:ml, mc, h, :],
                                start=(mc == 0),
                                stop=(mc == 1),
                            )
            av3 = av_ps[:, :, 0:390].rearrange("p m (h o) -> p m h o", o=65)
            recip_sb = sbr.tile([128, 2, 6], F32, name="recip_sb", tag="recip")
            nc.vector.reciprocal_approx_fast(
                out=recip_sb,
                in_=av3[:, :, :, 64],
            )
            nc.vector.tensor_tensor(
                y_sb,
                av3[:, :, :, 0:64],
                recip_sb[:, :, :, None].to_broadcast((128, 2, 6, 64)),
                ALU.mult,
            )
            return y_sb

        # persistent yT ring over global tokens: [128(c'), 3(p), 6272]
        # (6272 = lcm(196,128): blocks never wrap, chunks stay aligned)
        RING = 6272
        ring = singles.tile([128, 3, RING], BF16)
        ostate = {"fired": 0, "out_sb": None, "dq": []}

        def yT_to_ring(tb, y_sb):
            """transpose y -> ring at global token offset 196*tb."""
            ident = late_consts["ident"]
            g0 = (196 * tb) % RING
            yT_ps = ps2.tile([128, 1024], BF16, tag="ps2", name="yT_ps")
            for nc_i, (n0, nl) in enumerate(MC):
                for p in range(NP):
                    nc.tensor.transpose(
                        yT_ps[:, p * 196 + n0: p * 196 + n0 + nl],
                        y_sb[0:nl, nc_i, 2 * p: 2 * p + 2, :].rearrange(
                            "p a b -> p (a b)"
                        ),
                        ident[0:nl, 0:nl],
                    )
            nc.vector.tensor_copy(
                ring[:, :, g0: g0 + 196], yT_ps[:, 0:588].rearrange(
                    "p (a b) -> p a b", b=196
                )
            )

        def drain_chunk():
            """drain the oldest pending out chunk (deferred one chunk so
            DVE never head-of-line blocks on the chunk matmuls)."""
            c, o_ps = ostate["dq"].pop(0)
            bp_bc = late_consts["bp_bc"]
            _emit_chunk_drain(c, o_ps, bp_bc)

        def out_chunk(c):
            """out proj for global token chunk c (128 tokens)."""
            wp_sb = late_consts["wp_sb"]
            if ostate["out_sb"] is None:
                ostate["out_sb"] = sbout.tile([128, 7, 384], BF16, name="out_sb")
            g0 = (128 * c) % RING
            o_ps = ps1.tile([128, 512], F32, tag="ps1", name="o_ps")
            for p in range(NP):
                nc.tensor.matmul(
                    o_ps[:, 0:384],
                    ring[:, p, g0: g0 + 128],
                    wp_sb[:, p, :],
                    start=(p == 0),
                    stop=(p == 2),
                )
            ostate["dq"].append((c, o_ps))
            if len(ostate["dq"]) >= 2:
                drain_chunk()

        def _emit_chunk_drain(c, o_ps, bp_bc):
            nc.vector.tensor_tensor(
                ostate["out_sb"][:, c % 7, :],
                o_ps[:, 0:384],
                bp_bc,
                ALU.add,
            )
            last_chunk = (T * N) // 128 - 1
            if c == last_chunk - 1 and c % 7 == 5:
                # split the final group's store: ship the first 6 chunks now
                # so only the last 128 tokens trail the final drain
                grp = c // 7
                nc.sync.dma_start(
                    out2[grp * 896: grp * 896 + 768].rearrange(
                        "(c p) o -> p c o", p=128
                    ),
                    ostate["out_sb"][:, 0:6, :],
                )
            if c % 7 == 6:
                grp = c // 7
                if c == last_chunk:
                    nc.sync.dma_start(
                        out2[grp * 896 + 768: grp * 896 + 896].rearrange(
                            "(c p) o -> p c o", p=128
                        ),
                        ostate["out_sb"][:, 6:7, :],
                    )
                else:
                    nc.sync.dma_start(
                        out2[grp * 896: grp * 896 + 896].rearrange(
                            "(c p) o -> p c o", p=128
                        ),
                        ostate["out_sb"],
                    )
                ostate["out_sb"] = None

        def out_stage(tb, y_sb):
            yT_to_ring(tb, y_sb)
            # fire chunks one block late: decouples the chunk matmuls from
            # the just-issued ring copy
            avail = (196 * (tb - 1)) // 128 if tb >= 1 else 0
            while ostate["fired"] < avail:
                out_chunk(ostate["fired"])
                ostate["fired"] += 1

        # --- software-pipelined emission over block pairs ---
        # qk matmul groups interleaved with prev-block AV / out-proj so the
        # qk drains overlap PE work before the scores need them
        assert T % 2 == 0
        # qk is computed one pair AHEAD, its three groups interleaved with
        # the current pair's stages, so the psum drains never gate scores
        pendq = []  # [(attns, v_sb, tb)] - AV lags scores by TWO blocks
        cur = qk_start(0)  # (x8, qk_sb) for pair 0
        load_early_consts()
        load_late_consts()
        for p in range(3):
            qk_drain(cur[1], p, qk_mms(cur[0], p))
        for tp in range(T // 2):
            x8, qk_sb = cur
            nxt = qk_start(tp + 1) if tp + 1 < T // 2 else None

            old = pendq.pop(0) if len(pendq) >= 2 else None
            if old is not None:
                y_prev = av_stage(old[2], old[0], old[1])
            v_sb, v_ps = v_proj(2 * tp, x8, 0)
            attns = scores(2 * tp, qk_sb, 0)
            if nxt is not None:
                pss0 = qk_mms(nxt[0], 0)
            v_drain(v_sb, v_ps)
            if old is not None:
                out_stage(old[2], y_prev)
            if nxt is not None:
                qk_drain(nxt[1], 0, pss0)
            pendq.append((attns, v_sb, 2 * tp))

            # second block of the pair
            if nxt is not None:
                pss1 = qk_mms(nxt[0], 1)
            old = pendq.pop(0) if len(pendq) >= 2 else None
            if old is not None:
                y_prev = av_stage(old[2], old[0], old[1])
            if nxt is not None:
                qk_drain(nxt[1], 1, pss1)
                pss2 = qk_mms(nxt[0], 2)
            v_sb, v_ps = v_proj(2 * tp + 1, x8, 1)
            if nxt is not None:
                qk_drain(nxt[1], 2, pss2)
            attns = scores(2 * tp + 1, qk_sb, 1)
            v_drain(v_sb, v_ps)
            if old is not None:
                out_stage(old[2], y_prev)
            pendq.append((attns, v_sb, 2 * tp + 1))
            cur = nxt
        # drain the pipeline tail: both AVs, then both ring copies, then
        # every remaining chunk (most are ready before the last ring copy)
        ys = []
        for old in pendq:
            ys.append((old[2], av_stage(old[2], old[0], old[1])))
        for tb, y in ys:
            yT_to_ring(tb, y)
        while ostate["fired"] < (T * N) // 128:
            out_chunk(ostate["fired"])
            ostate["fired"] += 1
        while ostate["dq"]:
            drain_chunk()
        assert ostate["out_sb"] is None


def reference_numpy(x, W_qkv, b_qkv, W_proj, b_proj):
    """fp32 numpy port of the jax reference (for quick local checks)."""
    b, t, n, c = x.shape
    qkv = np.einsum("btnc,oc->btno", x, W_qkv) + b_qkv
    qkv = qkv.reshape(b, t, n, 3, NH, HD).transpose(3, 0, 4, 1, 2, 5)
    q, k, v = qkv[0] * SCALE, qkv[1], qkv[2]
    attn = np.einsum("bhtnd,bhtmd->bhtnm", q, k)
    attn = attn - attn.max(-1, keepdims=True)
    attn = np.exp(attn)
    attn = attn / attn.sum(-1, keepdims=True)
    out = np.einsum("bhtnm,bhtmd->bhtnd", attn, v)
    out = out.transpose(0, 2, 3, 4, 1).reshape(b, t, n, c)
    out = np.einsum("btnc,oc->btno", out, W_proj) + b_proj
    return out


_CACHE = {}


def _build_program():
    if "nc" in _CACHE:
        return _CACHE["nc"], _CACHE["t"]
    from concourse import bacc

    nc = bacc.Bacc("TRN2", target_bir_lowering=False, debug=False, num_devices=8)
    t = declare_tensors(nc, 64)
    with tile.TileContext(nc) as tc:
        build(tc, t, 64)
    nc.compile()
    _CACHE["nc"] = nc
    _CACHE["t"] = t
    return nc, t


def kernel(x, W_qkv, b_qkv, W_proj, b_proj):
    from concourse.bass_utils import run_bass_kernel_spmd

    x = np.asarray(x, dtype=np.float32)
    W_qkv = np.asarray(W_qkv, dtype=np.float32)
    b_qkv = np.asarray(b_qkv, dtype=np.float32)
    W_proj = np.asarray(W_proj, dtype=np.float32)
    b_proj = np.asarray(b_proj, dtype=np.float32)

    common, xs = prep_inputs(x, W_qkv, b_qkv, W_proj, b_proj)
    nc, _t = _build_program()
    in_maps = [dict(common, **xs[b]) for b in range(8)]
    res = run_bass_kernel_spmd(nc, in_maps, core_ids=list(range(8)))
    # out is a flat [T*N, 384] bf16 token stream per core
    outs = []
    for b in range(8):
        op = np.asarray(res.results[b]["out"]).astype(np.float32)
        outs.append(op.reshape(64, N, EMBED))
    return np.stack(outs)
